# revision 19
# baseline (speedup 1.0000x reference)
"""Multi-head attention TRN2 kernel (8-core SPMD, batch x head-half sharding).

Per core (c): batch b = c % 4, head-half g = c // 4 (8 of 16 heads).
Two passes of 4 heads each. Transposed-scores dataflow:
  QT/KT [hd, tok] head-major; V token-major with a ones column per head
  (softmax denominator falls out of the attn@V matmul, row 64).
All tensors bf16 (psum accumulation fp32). Normalization is per-q-block:
denominator reciprocals are broadcast across partitions with a tiny
PE matmul against a ones column (no DRAM roundtrip), and the output
projection chases each q-block. Pass-1 QKV projection chunks are
interleaved into pass-0's q loop so the shared psum pool round-robin
matches temporal order (keeps ACT fed at the pass boundary).
"""
import numpy as np
import concourse.bacc as bacc
import concourse.mybir as mybir
import concourse.tile as tile
from concourse import bass_utils

BF16 = mybir.dt.bfloat16
F32 = mybir.dt.float32
AF = mybir.ActivationFunctionType
NPBF16 = mybir.dt.np(mybir.dt.bfloat16)

S, D = 2048, 1024
CH = 512          # token chunk for stage A
NCH = S // CH     # 4
NKT = S // 128    # 16 key-token tiles
NQ = S // 512     # 4 query blocks of 512


def build(reps: int = 1, phases: str = "full"):
    nc = bacc.Bacc("TRN2", target_bir_lowering=False, debug=False, num_devices=8)
    xq_d = nc.dram_tensor("xq", [D, S], BF16, kind="ExternalInput")
    xk_d = nc.dram_tensor("xk", [D, S], BF16, kind="ExternalInput")
    xv_d = nc.dram_tensor("xv", [D, S], BF16, kind="ExternalInput")
    wq_d = nc.dram_tensor("wq", [D, 512], BF16, kind="ExternalInput")
    wk_d = nc.dram_tensor("wk", [D, 512], BF16, kind="ExternalInput")
    wv_d = nc.dram_tensor("wv", [D, 512], BF16, kind="ExternalInput")
    wo_d = nc.dram_tensor("wo", [512, D], BF16, kind="ExternalInput")
    bq_d = nc.dram_tensor("bq", [4, 128, 1], F32, kind="ExternalInput")
    bk_d = nc.dram_tensor("bk", [4, 128, 1], F32, kind="ExternalInput")
    bv_d = nc.dram_tensor("bv", [1, 512], F32, kind="ExternalInput")
    out_d = [
        nc.dram_tensor(f"out{p}", [S, D], BF16, kind="ExternalOutput") for p in (0, 1)
    ]

    with tile.TileContext(nc) as tc:
        with (
            tc.tile_pool(name="pers", bufs=1) as pers,
            tc.tile_pool(name="xp", bufs=2) as xp,
            tc.tile_pool(name="wp", bufs=1) as wp,
            tc.tile_pool(name="qkvp", bufs=4) as qkvp,
            tc.tile_pool(name="vpool", bufs=2) as vpool,
            tc.tile_pool(name="aop", bufs=4) as aop,
            tc.tile_pool(name="ptp", bufs=4) as ptp,
            tc.tile_pool(name="ocp", bufs=2) as ocp,
            tc.tile_pool(name="mmp", bufs=2, space="PSUM") as mmp,
            tc.tile_pool(name="stp", bufs=2, space="PSUM") as stp,
            tc.tile_pool(name="op", bufs=1, space="PSUM") as op,
        ):
            bq_sb = pers.tile([128, 4], F32, tag="bq")
            bk_sb = pers.tile([128, 4], F32, tag="bk")
            for m in range(4):
                nc.sync.dma_start(bq_sb[:, m : m + 1], bq_d[m])
                nc.sync.dma_start(bk_sb[:, m : m + 1], bk_d[m])
            bv_sb = pers.tile([128, 512], F32, tag="bv")
            nc.sync.dma_start(bv_sb[:], bv_d[:].to_broadcast((128, 512)))
            ones_sb = pers.tile([128, 64], BF16, tag="ones")
            nc.vector.memset(ones_sb[:], 1.0)

            def a_prep(ps: int):
                """Weights + tiles for pass ps's QKV projections."""
                wq_sb = wp.tile([128, 2048], BF16, tag="wq", name=f"wq{ps}")
                wk_sb = wp.tile([128, 2048], BF16, tag="wk", name=f"wk{ps}")
                wv_sb = wp.tile([128, 2048], BF16, tag="wv", name=f"wv{ps}")
                cs = slice(ps * 256, (ps + 1) * 256)
                for w_sb, w_d in ((wq_sb, wq_d), (wk_sb, wk_d), (wv_sb, wv_d)):
                    nc.sync.dma_start(
                        w_sb[:].rearrange("p (k m) -> p k m", k=8),
                        w_d[:, cs].rearrange("(k p) m -> p k m", p=128),
                    )
                QT = [
                    qkvp.tile([128, S], BF16, tag="qt", name=f"QT{ps}{pp}")
                    for pp in range(2)
                ]
                KT = [
                    qkvp.tile([128, S], BF16, tag="kt", name=f"KT{ps}{pp}")
                    for pp in range(2)
                ]
                v_sb = vpool.tile([128, NKT * 264], BF16, tag="v", name=f"v{ps}")
                for t in range(NKT):
                    vv = v_sb[:, t * 264 : (t + 1) * 264].rearrange(
                        "p (h c) -> p h c", h=4
                    )
                    nc.vector.memset(vv[:, :, 64:66], 0.0)
                    nc.vector.memset(vv[:, :, 64:65], 1.0)
                return wq_sb, wk_sb, wv_sb, QT, KT, v_sb

            def a_chunk(ps: int, ch: int, tiles):
                """One token chunk (CH tokens) of pass ps's QKV projections."""
                wq_sb, wk_sb, wv_sb, QT, KT, v_sb = tiles
                toks = slice(ch * CH, (ch + 1) * CH)
                xq_ch = xp.tile([128, 8 * CH], BF16, tag="xq", name=f"xq{ps}{ch}")
                xk_ch = xp.tile([128, 8 * CH], BF16, tag="xk", name=f"xk{ps}{ch}")
                xv_ch = xp.tile([128, 8 * CH], BF16, tag="xv", name=f"xv{ps}{ch}")
                for x_ch, x_d in ((xq_ch, xq_d), (xk_ch, xk_d), (xv_ch, xv_d)):
                    nc.scalar.dma_start(
                        x_ch[:].rearrange("p (k m) -> p k m", k=8),
                        x_d[:, toks].rearrange("(k p) m -> p k m", p=128),
                    )
                for pp in range(2):
                    m = 2 * ps + pp
                    pt_q = mmp.tile([128, 512], F32, tag="mm", name=f"aq{ps}{ch}{pp}")
                    pt_k = mmp.tile([128, 512], F32, tag="mm", name=f"ak{ps}{ch}{pp}")
                    for k in range(8):
                        lsl = slice(k * 256 + pp * 128, k * 256 + pp * 128 + 128)
                        ksl = slice(k * CH, (k + 1) * CH)
                        nc.tensor.matmul(
                            pt_q[:], wq_sb[:, lsl], xq_ch[:, ksl],
                            start=(k == 0), stop=(k == 7),
                        )
                        nc.tensor.matmul(
                            pt_k[:], wk_sb[:, lsl], xk_ch[:, ksl],
                            start=(k == 0), stop=(k == 7),
                        )
                    nc.vector.tensor_scalar_add(
                        QT[pp][:, toks], pt_q[:], bq_sb[:, m : m + 1]
                    )
                    nc.vector.tensor_scalar_add(
                        KT[pp][:, toks], pt_k[:], bk_sb[:, m : m + 1]
                    )
                for m2 in range(4):
                    pt_v = mmp.tile([128, 512], F32, tag="mm", name=f"av{ps}{ch}{m2}")
                    for k in range(8):
                        lsl = slice(k * CH + m2 * 128, k * CH + m2 * 128 + 128)
                        nc.tensor.matmul(
                            pt_v[:, 0:256],
                            xv_ch[:, lsl],
                            wv_sb[:, k * 256 : (k + 1) * 256],
                            start=(k == 0), stop=(k == 7),
                        )
                    t = ch * 4 + m2
                    dst = v_sb[
                        :, t * 264 : (t + 1) * 264
                    ].rearrange("p (h c) -> p h c", h=4)[:, :, 0:64]
                    src = pt_v[:, 0:256].rearrange("p (h c) -> p h c", h=4)
                    bvb = bv_sb[:, ps * 256 : (ps + 1) * 256].rearrange(
                        "p (h c) -> p h c", h=4
                    )
                    nc.vector.tensor_add(dst, src, bvb)

            def bc_prep(ps: int):
                sums = pers.tile([128, S], BF16, tag="sums", bufs=2, name=f"sums{ps}")
                nc.vector.memset(sums[:], 1.0)
                AT = [
                    aop.tile([128, S], BF16, tag="aot", name=f"AT{ps}{pp}")
                    for pp in range(2)
                ]
                wo_sb = wp.tile([128, 2048], BF16, tag="wo", bufs=2, name=f"wo{ps}")
                for kb in range(2):
                    rs = slice(ps * 256 + kb * 128, ps * 256 + kb * 128 + 128)
                    nc.sync.dma_start(wo_sb[:, kb * 1024 : (kb + 1) * 1024], wo_d[rs, :])
                return sums, AT, wo_sb

            def bc_attn(ps: int, q: int, a_tiles, b_tiles):
                """Attention + normalization for one q block."""
                _, _, _, QT, KT, v_sb = a_tiles
                sums, AT, wo_sb = b_tiles
                qsl = slice(q * 512, (q + 1) * 512)
                for pp in range(2):
                    oA = op.tile([66, 512], F32, tag="oA", name=f"oA{ps}{pp}{q}")
                    oB = op.tile([66, 512], F32, tag="oB", name=f"oB{ps}{pp}{q}")
                    for kt in range(NKT):
                        ksl = slice(kt * 128, (kt + 1) * 128)
                        st = stp.tile([128, 1024], F32, tag="st",
                                      name=f"st{ps}{pp}{q}{kt}")
                        nc.tensor.matmul(
                            st[:, 0:512], KT[pp][0:64, ksl], QT[pp][0:64, qsl],
                            start=True, stop=True,
                        )
                        nc.tensor.matmul(
                            st[:, 512:1024], KT[pp][64:128, ksl],
                            QT[pp][64:128, qsl], start=True, stop=True,
                        )
                        pt = ptp.tile([128, 1024], BF16, tag="pt",
                                      name=f"pt{ps}{pp}{q}{kt}")
                        nc.scalar.activation(pt[:], st[:], AF.Exp, scale=0.125)
                        nc.tensor.matmul(
                            oA[:],
                            v_sb[:, kt * 264 + (2 * pp) * 66 :
                                 kt * 264 + (2 * pp) * 66 + 66],
                            pt[:, 0:512],
                            start=(kt == 0), stop=(kt == NKT - 1),
                        )
                        nc.tensor.matmul(
                            oB[:],
                            v_sb[:, kt * 264 + (2 * pp + 1) * 66 :
                                 kt * 264 + (2 * pp + 1) * 66 + 66],
                            pt[:, 512:1024],
                            start=(kt == 0), stop=(kt == NKT - 1),
                        )
                    nc.vector.tensor_copy(
                        sums[64 * pp : 64 * pp + 1, qsl], oA[64:65, :]
                    )
                    nc.vector.tensor_copy(
                        sums[64 * pp + 32 : 64 * pp + 33, qsl], oB[64:65, :]
                    )
                    nc.vector.tensor_copy(AT[pp][0:64, qsl], oA[0:64, :])
                    nc.vector.tensor_copy(AT[pp][64:128, qsl], oB[0:64, :])
                with nc.allow_low_precision(reason="bf16 softmax denom, tol 2e-2"):
                    nc.vector.reciprocal(sums[:, qsl], sums[:, qsl])
                for pp in range(2):
                    rbc = mmp.tile([128, 512], F32, tag="mm", name=f"rbc{ps}{pp}{q}")
                    r0, r1 = 64 * pp, 64 * pp + 32
                    nc.tensor.matmul(
                        rbc[0:64, :], ones_sb[r0 : r0 + 1, :],
                        sums[r0 : r0 + 1, qsl],
                        start=True, stop=True, tile_position=(r0, 0),
                    )
                    nc.tensor.matmul(
                        rbc[64:128, :], ones_sb[r1 : r1 + 1, :],
                        sums[r1 : r1 + 1, qsl],
                        start=True, stop=True, tile_position=(r1, 64),
                    )
                    nc.vector.tensor_mul(
                        AT[pp][:, qsl], AT[pp][:, qsl], rbc[:]
                    )

            def bc_proj(ps: int, q: int, b_tiles):
                """Output projection for one q block (4 token blocks)."""
                _, AT, wo_sb = b_tiles
                for mp in range(4 * q, 4 * q + 4):
                    msl = slice(mp * 128, (mp + 1) * 128)
                    oc = ocp.tile([128, 1024], BF16, tag="oc", name=f"oc{ps}{mp}")
                    for n in range(2):
                        ps_t = mmp.tile([128, 512], F32, tag="mm",
                                        name=f"c{ps}{mp}{n}")
                        for kb in range(2):
                            nc.tensor.matmul(
                                ps_t[:],
                                AT[kb][:, msl],
                                wo_sb[:, kb * 1024 + n * 512 :
                                      kb * 1024 + n * 512 + 512],
                                start=(kb == 0), stop=(kb == 1),
                            )
                        nc.vector.tensor_copy(
                            oc[:, n * 512 : (n + 1) * 512], ps_t[:]
                        )
                    nc.sync.dma_start(out_d[ps][msl, :], oc[:])

            def body():
                t0 = a_prep(0)
                for ch in range(NCH):
                    a_chunk(0, ch, t0)
                t1 = a_prep(1)
                b0 = bc_prep(0)
                bc_attn(0, 0, t0, b0)
                for q in range(1, NQ):
                    bc_attn(0, q, t0, b0)
                    a_chunk(1, q - 1, t1)
                    bc_proj(0, q - 1, b0)
                a_chunk(1, NQ - 1, t1)
                bc_proj(0, NQ - 1, b0)
                b1 = bc_prep(1)
                bc_attn(1, 0, t1, b1)
                for q in range(1, NQ):
                    bc_attn(1, q, t1, b1)
                    bc_proj(1, q - 1, b1)
                bc_proj(1, NQ - 1, b1)

            if reps == 0:
                body()
            else:
                with tc.For_i(0, reps, 1):
                    body()

    nc.compile()
    return nc


def make_in_maps(query, key, value, Wq, bq, Wk, bk, Wv, bv, Wo, bo):
    """Host-side sharding: per-core input dicts (8 cores)."""
    qT = [np.ascontiguousarray(query[b].T).astype(NPBF16) for b in range(4)]
    kT = [np.ascontiguousarray(key[b].T).astype(NPBF16) for b in range(4)]
    vT = [np.ascontiguousarray(value[b].T).astype(NPBF16) for b in range(4)]
    in_maps = []
    for c in range(8):
        b, g = c % 4, c // 4
        hs = slice(g * 512, (g + 1) * 512)
        in_maps.append(
            {
                "xq": qT[b],
                "xk": kT[b],
                "xv": vT[b],
                "wq": np.ascontiguousarray(Wq[hs, :].T).astype(NPBF16),
                "wk": np.ascontiguousarray(Wk[hs, :].T).astype(NPBF16),
                "wv": np.ascontiguousarray(Wv[hs, :].T).astype(NPBF16),
                "wo": np.ascontiguousarray(Wo[:, hs].T).astype(NPBF16),
                "bq": np.ascontiguousarray(bq[hs].reshape(4, 128, 1), np.float32),
                "bk": np.ascontiguousarray(bk[hs].reshape(4, 128, 1), np.float32),
                "bv": np.ascontiguousarray(bv[hs].reshape(1, 512), np.float32),
            }
        )
    return in_maps


def assemble(results, bo):
    """Sum partials: out[b] = sum over half g, pass p of core partials + bo."""
    out = np.zeros((4, S, D), np.float32)
    for c in range(8):
        b = c % 4
        out[b] += results[c]["out0"].astype(np.float32)
        out[b] += results[c]["out1"].astype(np.float32)
    out += np.asarray(bo, np.float32)[None, None, :]
    return out


_NC_CACHE = {}


def kernel(query, key, value, Wq, bq, Wk, bk, Wv, bv, Wo, bo, *, nc=None):
    query = np.asarray(query, np.float32)
    key = np.asarray(key, np.float32)
    value = np.asarray(value, np.float32)
    in_maps = make_in_maps(query, key, value, Wq, bq, Wk, bk, Wv, bv, Wo, bo)
    if nc is None:
        if "nc" not in _NC_CACHE:
            _NC_CACHE["nc"] = build(reps=0)
        nc = _NC_CACHE["nc"]
    res = bass_utils.run_bass_kernel_spmd(nc, in_maps, core_ids=list(range(8)))
    return assemble(res.results, bo)
